# revision 1
# baseline (speedup 1.0000x reference)
"""Self-contained Trainium2 Bass kernel for GQA causal self-attention.

Problem: x[2,2048,4096] @ wq/wk/wv (32 q-heads, 8 kv-heads, head_dim 128),
rope (precomputed freqs), causal softmax, GQA attention, wo projection.

Sharding: tensor-parallel across heads over 8 NeuronCores -- core g gets
kv-head g and q-heads 4g..4g+3 (wq/wk/wv column-sharded, wo row-sharded).
Each core computes a partial output projection; the host sums the 8
partials and transposes back (wo is row-parallel so partials just add).

See build() for the 3-phase device pipeline. All matmuls run as float32r
(TF32-like, ~1.5e-4 rel err, full 1-cycle/row PE rate).
"""
import numpy as np
import concourse.bacc as bacc
import concourse.mybir as mybir
import concourse.tile as tile

F32 = mybir.dt.float32
F32R = mybir.dt.float32r
AF = mybir.ActivationFunctionType
OP = mybir.AluOpType

P = 128
B, S, D = 2, 2048, 4096
T = B * S            # 4096 tokens
HD = 128             # head dim
NQ = 4               # q heads per core
DC = D // P          # 32 contraction chunks
NT = 512             # free-dim tile
TT = T // NT         # 8 token tiles
SKC = S // P         # 16 s_k chunks per batch
SQT = S // NT        # 4 s_q tiles per batch
KG = 8               # k-chunk groups (4 chunks each) in phase 1
SCALE = 1.0 / float(np.sqrt(HD))
NEG = -1.0e9


def build():
    nc = bacc.Bacc("TRN2", target_bir_lowering=False)
    # pre-shuffled inputs (see host_inputs)
    xh = nc.dram_tensor("xh", [TT, KG, P, 4, NT], F32R, kind="ExternalInput")
    wqh = nc.dram_tensor("wqh", [P, DC, NQ * HD], F32R, kind="ExternalInput")
    wkh = nc.dram_tensor("wkh", [P, DC, HD], F32R, kind="ExternalInput")
    wvh = nc.dram_tensor("wvh", [P, DC, HD], F32R, kind="ExternalInput")
    woh = nc.dram_tensor("woh", [TT, P, NQ, NT], F32R, kind="ExternalInput")
    cosE = nc.dram_tensor("cosE", [P, T], F32, kind="ExternalInput")
    sinE = nc.dram_tensor("sinE", [P, T], F32, kind="ExternalInput")
    perm = nc.dram_tensor("perm", [P, P], F32R, kind="ExternalInput")
    ident = nc.dram_tensor("ident", [P, P], F32R, kind="ExternalInput")
    ones = nc.dram_tensor("ones", [P, P], F32R, kind="ExternalInput")
    maskneg = nc.dram_tensor("maskneg", [P, NT // P, NT], F32, kind="ExternalInput")
    outT = nc.dram_tensor("outT", [D, T], F32, kind="ExternalOutput")

    with tile.TileContext(nc) as tc:
        with (
            tc.tile_pool(name="kvres", bufs=1) as kvres,
            tc.tile_pool(name="dram", bufs=1, space="DRAM") as dpool,
        ):
            kT_res = kvres.tile([P, T], F32R)          # roped K^T, resident
            v_res = kvres.tile([P, DC, HD], F32R)      # V natural (chunked), resident
            qs = kvres   # qt tiles carved top-level so prefetch crosses barriers
            ats = kvres  # at tiles likewise (phase-3 prefetch during phase 2)
            ones_sb = kvres.tile([P, P], F32R)
            nc.scalar.dma_start(ones_sb[:], ones[:, :])
            mask_sb = kvres.tile([P, NT // P, NT], F32)
            nc.scalar.dma_start(mask_sb[:], maskneg[:, :, :])
            qT_d = dpool.tile([NQ, HD, T], F32R)       # roped Q^T spill
            attnT_d = dpool.tile([TT, HD, NQ, NT], F32R)  # normalized attn^T spill

            # ---------------- phase 1: projections + rope ----------------
            with (
                tc.tile_pool(name="c1", bufs=1) as c1,
                tc.tile_pool(name="wgt", bufs=1) as wgt,
                tc.tile_pool(name="xs", bufs=3) as xs,
                tc.tile_pool(name="epi", bufs=2) as epi,
                tc.tile_pool(name="ps_acc", bufs=7, space="PSUM") as ps_acc,
                tc.tile_pool(name="ps_misc", bufs=1, space="PSUM") as ps_misc,
            ):
                perm_sb = c1.tile([P, P], F32R)
                nc.scalar.dma_start(perm_sb[:], perm[:, :])
                ident_sb = c1.tile([P, P], F32R)
                nc.scalar.dma_start(ident_sb[:], ident[:, :])
                # weights interleaved by k-group so tt0 matmuls start early
                wq_sb = c1.tile([P, DC, NQ * HD], F32R)
                wk_sb = wgt.tile([P, DC, HD], F32R)
                wv_sb = wgt.tile([P, DC, HD], F32R)

                for tt in range(TT):
                    tsl = slice(tt * NT, (tt + 1) * NT)
                    cos_t = xs.tile([P, NT], F32, tag="cos", bufs=2)
                    sin_t = xs.tile([P, NT], F32, tag="sin", bufs=2)
                    nc.sync.dma_start(cos_t[:], cosE[:, tsl])
                    nc.sync.dma_start(sin_t[:], sinE[:, tsl])

                    accs = [ps_acc.tile([P, NT], F32, tag="acc", name=f"acc{tt}_{oc}")
                            for oc in range(6)]
                    for kg in range(KG):
                        if tt == 0:
                            ksl = slice(kg * 4, (kg + 1) * 4)
                            nc.sync.dma_start(wq_sb[:, ksl, :], wqh[:, ksl, :])
                            nc.sync.dma_start(wk_sb[:, ksl, :], wkh[:, ksl, :])
                            nc.sync.dma_start(wv_sb[:, ksl, :], wvh[:, ksl, :])
                        xt = xs.tile([P, 4, NT], F32R, tag="xt", bufs=3)
                        nc.sync.dma_start(xt[:], xh[tt, kg, :, :, :])
                        for kc in range(4):
                            k = kg * 4 + kc
                            for oc in range(6):
                                if oc < 4:
                                    lhsT = wq_sb[:, k, oc * P:(oc + 1) * P]
                                elif oc == 4:
                                    lhsT = wk_sb[:, k, :]
                                else:
                                    lhsT = wv_sb[:, k, :]
                                nc.tensor.matmul(accs[oc][:], lhsT=lhsT,
                                                 rhs=xt[:, kc, :],
                                                 start=(k == 0),
                                                 stop=(k == DC - 1))

                    # rope epilogue for Q (oc 0..3) and K (oc 4)
                    for oc in range(5):
                        acc = accs[oc]
                        sb_r = epi.tile([P, NT], F32R, tag="sbr")
                        nc.scalar.copy(sb_r[:], acc[:])
                        sw = ps_misc.tile([P, NT], F32, tag="misc", name="sw")
                        nc.tensor.matmul(sw[:], lhsT=perm_sb[:], rhs=sb_r[:],
                                         start=True, stop=True)
                        t1 = epi.tile([P, NT], F32, tag="t1")
                        nc.vector.tensor_tensor(t1[:], acc[:], cos_t[:], op=OP.mult)
                        t2 = epi.tile([P, NT], F32, tag="t2")
                        nc.vector.tensor_tensor(t2[:], sw[:], sin_t[:], op=OP.mult)
                        if oc < 4:
                            qro = epi.tile([P, NT], F32R, tag="qro")
                            nc.vector.tensor_tensor(qro[:], t1[:], t2[:], op=OP.add)
                            nc.gpsimd.dma_start(qT_d[oc, :, tsl], qro[:])
                        else:
                            nc.vector.tensor_tensor(kT_res[:, tsl], t1[:], t2[:],
                                                    op=OP.add)

                    # V epilogue: V^T chunk -> transpose -> V resident
                    vsb = epi.tile([P, NT], F32R, tag="sbr")
                    nc.scalar.copy(vsb[:], accs[5][:])
                    for c in range(NT // P):
                        vt_ps = ps_misc.tile([P, NT], F32R, tag="misc",
                                             name="vt_ps")[:, 0:P]
                        nc.tensor.transpose(vt_ps[:], vsb[:, c * P:(c + 1) * P],
                                            ident_sb[:])
                        nc.scalar.copy(v_res[:, 4 * tt + c, :],
                                       vt_ps[:].bitcast(F32))

            with tc.tile_pool(name="wos", bufs=1) as wos:
                wo_sb = wos.tile([P, TT, NQ, NT], F32R)

                # ---------------- phase 2: attention ----------------
                with (
                    tc.tile_pool(name="pts", bufs=3) as pts,
                    tc.tile_pool(name="ep2", bufs=2) as ep2,
                    tc.tile_pool(name="ps_st", bufs=3, space="PSUM") as ps_st,
                    tc.tile_pool(name="ps_att", bufs=1, space="PSUM") as ps_att,
                    tc.tile_pool(name="ps_den", bufs=1, space="PSUM") as ps_den,
                ):
                    for b in range(B):
                        for h in range(NQ):
                            # background wo prefetch, one chunk per (b, h)
                            wj = b * NQ + h
                            nc.scalar.dma_start(wo_sb[:, wj, :, :],
                                                woh[wj, :, :, :])
                            qt_sb = qs.tile([P, S], F32R, tag="qt", bufs=2)
                            nc.sync.dma_start(qt_sb[:],
                                              qT_d[h, :, b * S:(b + 1) * S])
                            for jq in range(SQT):
                                nk = 4 * (jq + 1)
                                att_ps = ps_att.tile([P, NT], F32, tag="attn")
                                den_ps = ps_den.tile([P, NT], F32, tag="den")
                                qsl = slice(jq * NT, (jq + 1) * NT)
                                for ip in range(nk // 2):   # ik pairs
                                    st = ps_st.tile([P, 2, NT], F32, tag="st")
                                    for half in range(2):
                                        ik = 2 * ip + half
                                        nc.tensor.matmul(
                                            st[:, half, :],
                                            lhsT=kT_res[:, b * S + ik * P:
                                                        b * S + (ik + 1) * P],
                                            rhs=qt_sb[:, qsl],
                                            start=True, stop=True)
                                    pt = pts.tile([P, 2, NT], F32R, tag="pt")
                                    r = 2 * ip - 4 * jq
                                    if r >= 0:
                                        pr = pts.tile([P, 2, NT], F32R, tag="pr")
                                        nc.scalar.activation(pr[:, :, :],
                                                             st[:, :, :],
                                                             AF.Exp, scale=SCALE)
                                        nc.vector.tensor_tensor(
                                            pt[:, :, :], pr[:, :, :].bitcast(F32),
                                            mask_sb[:, r:r + 2, :], op=OP.mult)
                                    else:
                                        nc.scalar.activation(pt[:, :, :],
                                                             st[:, :, :],
                                                             AF.Exp, scale=SCALE)
                                    for half in range(2):
                                        ik = 2 * ip + half
                                        nc.tensor.matmul(
                                            att_ps[:],
                                            lhsT=v_res[:, 16 * b + ik, :],
                                            rhs=pt[:, half, :],
                                            start=(ik == 0), stop=(ik == nk - 1))
                                        nc.tensor.matmul(
                                            den_ps[:], lhsT=ones_sb[:],
                                            rhs=pt[:, half, :],
                                            start=(ik == 0), stop=(ik == nk - 1))
                                rc = ep2.tile([P, NT], F32, tag="rc")
                                nc.vector.reciprocal_approx_fast(rc[:], den_ps[:])
                                an = ep2.tile([P, NT], F32R, tag="an")
                                nc.vector.tensor_tensor(an[:], att_ps[:], rc[:],
                                                        op=OP.mult)
                                nc.gpsimd.dma_start(
                                    attnT_d[b * SQT + jq, :, h, :], an[:])

                # ---------------- phase 3: output projection ----------------
                with (
                    tc.tile_pool(name="outp", bufs=8) as outp,
                    tc.tile_pool(name="ps_o", bufs=4, space="PSUM") as ps_o,
                ):
                    for jt in range(TT):
                        jsl = slice(jt * NT, (jt + 1) * NT)
                        at = ats.tile([P, NQ, NT], F32R, tag="qt", bufs=2)
                        nc.scalar.dma_start(at[:], attnT_d[jt, :, :, :])
                        for oc in range(D // P):
                            o_ps = ps_o.tile([P, NT], F32, tag="o")
                            wj, wn = oc // 4, (oc % 4) * P
                            for dc in range(NQ):
                                nc.tensor.matmul(
                                    o_ps[:],
                                    lhsT=wo_sb[:, wj, dc, wn:wn + P],
                                    rhs=at[:, dc, :],
                                    start=(dc == 0), stop=(dc == NQ - 1))
                            osb = outp.tile([P, NT], F32, tag="ot")
                            nc.scalar.copy(osb[:], o_ps[:])
                            nc.sync.dma_start(
                                outT[oc * P:(oc + 1) * P, jsl], osb[:])

    nc.compile()
    return nc


def host_inputs(x, wq, wk, wv, wo, freqs_cos, freqs_sin):
    """Build the 8 per-core input maps from full inputs (pre-shuffled)."""
    x2 = np.asarray(x, dtype=np.float32).reshape(T, D)
    # xh[tt, kg, p, kc, n] = x2[tt*NT + n, kg*512 + kc*128 + p]
    xh = np.ascontiguousarray(
        x2.reshape(TT, NT, KG, 4, P).transpose(0, 2, 4, 3, 1))
    fc = np.asarray(freqs_cos, dtype=np.float32)
    fs = np.asarray(freqs_sin, dtype=np.float32)
    cc = np.repeat(fc.T, 2, axis=0)                         # [128, S]
    ss = np.repeat(fs.T, 2, axis=0)
    sgn = np.ones((P, 1), np.float32)
    sgn[0::2, 0] = -1.0
    cosE = np.ascontiguousarray(np.tile(cc, (1, B)).astype(np.float32))
    sinE = np.ascontiguousarray(np.tile(ss * sgn, (1, B)).astype(np.float32))
    perm_np = np.zeros((P, P), np.float32)
    for i in range(P):
        perm_np[i, i ^ 1] = 1.0
    ident_np = np.eye(P, dtype=np.float32)
    ones_np = np.ones((P, P), np.float32)
    mk = np.zeros((P, NT // P, NT), np.float32)
    for r in range(NT // P):
        for p in range(P):
            mk[p, r, :] = np.where(np.arange(NT) >= 128 * r + p, 1.0, 0.0)

    wq_f = np.asarray(wq, dtype=np.float32)
    wk_f = np.asarray(wk, dtype=np.float32)
    wv_f = np.asarray(wv, dtype=np.float32)
    wo_f = np.asarray(wo, dtype=np.float32)
    in_maps = []
    for g in range(8):
        wq_g = wq_f[:, g * NQ * HD:(g + 1) * NQ * HD]       # [D, 512]
        wk_g = wk_f[:, g * HD:(g + 1) * HD]                 # [D, 128]
        wv_g = wv_f[:, g * HD:(g + 1) * HD]
        wo_g = wo_f[g * NQ * HD:(g + 1) * NQ * HD, :]       # [512, D]
        # [P, DC, M] with element [p, c, m] = w[c*128 + p, m]
        wqh_np = np.ascontiguousarray(
            wq_g.reshape(DC, P, NQ * HD).transpose(1, 0, 2))
        wkh_np = np.ascontiguousarray(wk_g.reshape(DC, P, HD).transpose(1, 0, 2))
        wvh_np = np.ascontiguousarray(wv_g.reshape(DC, P, HD).transpose(1, 0, 2))
        # woh[j, p, dc, n] = wo_g[dc*128 + p, j*512 + n]
        woh_np = np.ascontiguousarray(
            wo_g.reshape(NQ, P, TT, NT).transpose(2, 1, 0, 3))
        in_maps.append({
            "xh": xh, "wqh": wqh_np, "wkh": wkh_np, "wvh": wvh_np,
            "woh": woh_np,
            "cosE": cosE, "sinE": sinE, "perm": perm_np, "ident": ident_np,
            "ones": ones_np, "maskneg": mk,
        })
    return in_maps


def combine_outputs(results):
    """Sum per-core partial^T and transpose back to [B, S, D]."""
    acc = results[0]["outT"].astype(np.float32).copy()
    for r in results[1:]:
        acc += r["outT"]
    return np.ascontiguousarray(acc.T).reshape(B, S, D).astype(np.float32)


_NC = None


def kernel(x, wq, wk, wv, wo, freqs_cos, freqs_sin):
    """Full-input entry point: shards across 8 cores, runs, gathers."""
    global _NC
    from concourse.bass_utils import run_bass_kernel_spmd
    if _NC is None:
        _NC = build()
    in_maps = host_inputs(x, wq, wk, wv, wo, freqs_cos, freqs_sin)
    res = run_bass_kernel_spmd(_NC, in_maps, core_ids=list(range(8)),
                               trace=False)
    return combine_outputs(res.results)



# revision 11
# speedup vs baseline: 1.1145x; 1.1145x over previous
"""Self-contained Trainium2 Bass kernel for GQA causal self-attention.

Problem: x[2,2048,4096] @ wq/wk/wv (32 q-heads, 8 kv-heads, head_dim 128),
rope (precomputed freqs), causal softmax, GQA attention, wo projection.

Sharding: tensor-parallel across heads over 8 NeuronCores -- core g gets
kv-head g and q-heads 4g..4g+3 (wq/wk/wv column-sharded, wo row-sharded).
Each core computes a partial output projection; the host sums the 8
partials and transposes back (wo is row-parallel so partials just add).

v2: all tensors bf16 (halves DMA + enables FWL weight loads), Q^T and
attention outputs SBUF-resident (no DRAM spills), output projection fused
per 512-token block (software-pipelined one block behind attention), and
softmax denominators accumulated on the vector engine (bf16 adds) with
two small PE matmuls per (block, head) instead of one per k-chunk.
"""
import numpy as np
import ml_dtypes
import concourse.bacc as bacc
import concourse.mybir as mybir
import concourse.tile as tile

F32 = mybir.dt.float32
BF16 = mybir.dt.bfloat16
AF = mybir.ActivationFunctionType
OP = mybir.AluOpType

P = 128
B, S, D = 2, 2048, 4096
T = B * S            # 4096 tokens
HD = 128             # head dim
NQ = 4               # q heads per core
DC = D // P          # 32 contraction chunks
NT = 512             # free-dim tile
TT = T // NT         # 8 token tiles
SQT = S // NT        # 4 s_q tiles per batch
KG = 8               # k-chunk groups (4 chunks each) in phase 1
SCALE = 1.0 / float(np.sqrt(HD))


def build():
    nc = bacc.Bacc("TRN2", target_bir_lowering=False)
    # pre-shuffled inputs (see host_inputs)
    xh = nc.dram_tensor("xh", [TT, KG, P, 4, NT], BF16, kind="ExternalInput")
    wqh = nc.dram_tensor("wqh", [P, DC, NQ * HD], BF16, kind="ExternalInput")
    wkh = nc.dram_tensor("wkh", [P, DC, HD], BF16, kind="ExternalInput")
    wvh = nc.dram_tensor("wvh", [P, DC, HD], BF16, kind="ExternalInput")
    woh = nc.dram_tensor("woh", [P, TT, NQ, NT], BF16, kind="ExternalInput")
    cosE = nc.dram_tensor("cosE", [P, T], F32, kind="ExternalInput")
    sinE = nc.dram_tensor("sinE", [P, T], F32, kind="ExternalInput")
    perm = nc.dram_tensor("perm", [P, P], BF16, kind="ExternalInput")
    ident = nc.dram_tensor("ident", [P, P], BF16, kind="ExternalInput")
    ones = nc.dram_tensor("ones", [P, P], BF16, kind="ExternalInput")
    maskneg = nc.dram_tensor("maskneg", [P, NT // P, NT], BF16,
                             kind="ExternalInput")
    outT = nc.dram_tensor("outT", [D, T], BF16, kind="ExternalOutput")

    with tile.TileContext(nc) as tc:
        with (
            tc.tile_pool(name="res", bufs=1) as res,
            tc.tile_pool(name="outp", bufs=8) as outp,
        ):
            kT_res = res.tile([P, T], BF16)           # roped K^T, resident
            v_res = res.tile([P, DC, HD], BF16)       # V natural, resident
            qT_res = res.tile([P, NQ, T], BF16)       # roped Q^T, resident
            wo_sb = res.tile([P, TT, NQ, NT], BF16)   # wo, resident
            nc.scalar.dma_start(wo_sb[:], woh[:, :, :, :])
            ones_sb = res.tile([P, P], BF16)
            nc.scalar.dma_start(ones_sb[:], ones[:, :])
            mask_sb = res.tile([P, NT // P, NT], BF16)
            nc.scalar.dma_start(mask_sb[:], maskneg[:, :, :])

            # ---------------- phase 1: projections + rope ----------------
            with (
                tc.tile_pool(name="c1", bufs=1) as c1,
                tc.tile_pool(name="xs", bufs=3) as xs,
                tc.tile_pool(name="epi", bufs=2) as epi,
                tc.tile_pool(name="ps_acc", bufs=7, space="PSUM") as ps_acc,
                tc.tile_pool(name="ps_misc", bufs=1, space="PSUM") as ps_misc,
            ):
                perm_sb = c1.tile([P, P], BF16)
                nc.scalar.dma_start(perm_sb[:], perm[:, :])
                # weights interleaved by k-group so tt0 matmuls start early
                wq_sb = c1.tile([P, DC, NQ * HD], BF16)
                wk_sb = c1.tile([P, DC, HD], BF16)
                wv_sb = c1.tile([P, DC, HD], BF16)

                for tt in range(TT):
                    tsl = slice(tt * NT, (tt + 1) * NT)
                    cos_t = xs.tile([P, NT], F32, tag="cos", bufs=2)
                    sin_t = xs.tile([P, NT], F32, tag="sin", bufs=2)
                    nc.scalar.dma_start(cos_t[:], cosE[:, tsl])
                    nc.scalar.dma_start(sin_t[:], sinE[:, tsl])

                    accs = [ps_acc.tile([P, NT], F32, tag="acc",
                                        name=f"acc{tt}_{oc}")
                            for oc in range(6)]
                    v_ps = accs[5]
                    for kg in range(KG):
                        if tt == 0:
                            ksl = slice(kg * 4, (kg + 1) * 4)
                            nc.gpsimd.dma_start(wq_sb[:, ksl, :],
                                                wqh[:, ksl, :])
                            nc.gpsimd.dma_start(wk_sb[:, ksl, :],
                                                wkh[:, ksl, :])
                            nc.gpsimd.dma_start(wv_sb[:, ksl, :],
                                                wvh[:, ksl, :])
                        xt = xs.tile([P, 4, NT], BF16, tag="xt", bufs=3)
                        nc.sync.dma_start(xt[:], xh[tt, kg, :, :, :])
                        for kc in range(4):
                            k = kg * 4 + kc
                            for oc in range(6):
                                if oc < 4:
                                    lhsT = wq_sb[:, k, oc * P:(oc + 1) * P]
                                elif oc == 4:
                                    lhsT = wk_sb[:, k, :]
                                else:
                                    lhsT = wv_sb[:, k, :]
                                nc.tensor.matmul(accs[oc][:], lhsT=lhsT,
                                                 rhs=xt[:, kc, :],
                                                 start=(k == 0),
                                                 stop=(k == DC - 1))

                    # rope epilogue for Q (oc 0..3) and K (oc 4)
                    for oc in range(5):
                        acc = accs[oc]
                        sb_r = epi.tile([P, NT], BF16, tag="sbr")
                        nc.scalar.copy(sb_r[:], acc[:])
                        sw = ps_misc.tile([P, NT], F32, tag="misc", name="sw")
                        nc.tensor.matmul(sw[:], lhsT=perm_sb[:], rhs=sb_r[:],
                                         start=True, stop=True)
                        t1 = epi.tile([P, NT], F32, tag="t1")
                        nc.vector.tensor_tensor(t1[:], acc[:], cos_t[:],
                                                op=OP.mult)
                        t2 = epi.tile([P, NT], F32, tag="t2")
                        nc.vector.tensor_tensor(t2[:], sw[:], sin_t[:],
                                                op=OP.mult)
                        if oc < 4:
                            nc.vector.tensor_tensor(qT_res[:, oc, tsl],
                                                    t1[:], t2[:], op=OP.add)
                        else:
                            nc.vector.tensor_tensor(kT_res[:, tsl],
                                                    t1[:], t2[:], op=OP.add)

                    # V epilogue: V^T chunks -> DMA xbar transpose -> V
                    vsb = epi.tile([P, NT], BF16, tag="sbr")
                    nc.scalar.copy(vsb[:], v_ps[:])
                    for c in range(NT // P):
                        nc.sync.dma_start(v_res[:, 4 * tt + c, :],
                                          vsb[:, c * P:(c + 1) * P],
                                          transpose=True)

            # ---------- phase 2+3 fused: attention + output proj ----------
            with (
                tc.tile_pool(name="pts", bufs=3) as pts,
                tc.tile_pool(name="ep2", bufs=2) as ep2,
                tc.tile_pool(name="ats", bufs=2) as ats,
                tc.tile_pool(name="ps_st", bufs=2, space="PSUM") as ps_st,
                tc.tile_pool(name="ps_att", bufs=2, space="PSUM") as ps_att,
                tc.tile_pool(name="ps_o", bufs=2, space="PSUM") as ps_o,
            ):
                def emit_sweep(jt, at):
                    jsl = slice(jt * NT, (jt + 1) * NT)
                    for oc in range(DC):
                        o_ps = ps_o.tile([P, NT], F32, tag="o")
                        wj, wn = oc // 4, (oc % 4) * P
                        for dc in range(NQ):
                            nc.tensor.matmul(o_ps[:],
                                             lhsT=wo_sb[:, wj, dc, wn:wn + P],
                                             rhs=at[:, dc, :],
                                             start=(dc == 0),
                                             stop=(dc == NQ - 1))
                        osb = outp.tile([P, NT], BF16, tag="ot")
                        if oc % 2 == 0:
                            nc.scalar.copy(osb[:], o_ps[:])
                        else:
                            nc.vector.tensor_copy(osb[:], o_ps[:])
                        nc.sync.dma_start(outT[oc * P:(oc + 1) * P, jsl],
                                          osb[:])

                prev = None
                # jq descending: the first block (no prior sweep to hide
                # exp latency behind) gets the deepest matmul pipeline
                for b in range(B):
                    for jq in sorted(range(SQT), reverse=(b == 0)):
                        nk = 4 * (jq + 1)
                        qsl = slice(b * S + jq * NT, b * S + (jq + 1) * NT)
                        at = ats.tile([P, NQ, NT], BF16, tag="at")
                        for h in range(NQ):
                            att_ps = ps_att.tile([P, NT], F32, tag="att")
                            acc = pts.tile([P, 2, NT], BF16, tag="acc",
                                           bufs=2)
                            for ip in range(nk // 2):
                                st = ps_st.tile([P, 2, NT], F32, tag="st")
                                for half in range(2):
                                    ik = 2 * ip + half
                                    nc.tensor.matmul(
                                        st[:, half, :],
                                        lhsT=kT_res[:, b * S + ik * P:
                                                    b * S + (ik + 1) * P],
                                        rhs=qT_res[:, h, qsl],
                                        start=True, stop=True)
                                dst = acc if ip == 0 else pts.tile(
                                    [P, 2, NT], BF16, tag="pt", bufs=3)
                                r = 2 * ip - 4 * jq
                                if r >= 0:
                                    pr = pts.tile([P, 2, NT], BF16,
                                                  tag="pr", bufs=2)
                                    nc.scalar.activation(pr[:, :, :],
                                                         st[:, :, :],
                                                         AF.Exp, scale=SCALE)
                                    nc.vector.tensor_tensor(
                                        dst[:, :, :], pr[:, :, :],
                                        mask_sb[:, r:r + 2, :], op=OP.mult)
                                else:
                                    nc.scalar.activation(dst[:, :, :],
                                                         st[:, :, :],
                                                         AF.Exp, scale=SCALE)
                                if ip > 0:
                                    nc.vector.tensor_tensor(
                                        acc[:, :, :], acc[:, :, :],
                                        dst[:, :, :], op=OP.add)
                                for half in range(2):
                                    ik = 2 * ip + half
                                    nc.tensor.matmul(
                                        att_ps[:],
                                        lhsT=v_res[:, 16 * b + ik, :],
                                        rhs=dst[:, half, :],
                                        start=(ik == 0), stop=(ik == nk - 1))
                            den_ps = ps_o.tile([P, NT], F32, tag="o")
                            for half in range(2):
                                nc.tensor.matmul(den_ps[:], lhsT=ones_sb[:],
                                                 rhs=acc[:, half, :],
                                                 start=(half == 0),
                                                 stop=(half == 1))
                            rc = ep2.tile([P, NT], F32, tag="rc")
                            nc.vector.reciprocal_approx_fast(rc[:], den_ps[:])
                            nc.vector.tensor_tensor(at[:, h, :], att_ps[:],
                                                    rc[:], op=OP.mult)
                        if prev is not None:
                            emit_sweep(*prev)
                        prev = (b * SQT + jq, at)
                emit_sweep(*prev)

    nc.compile()
    return nc


def host_inputs(x, wq, wk, wv, wo, freqs_cos, freqs_sin):
    """Build the 8 per-core input maps from full inputs (pre-shuffled)."""
    bf16 = ml_dtypes.bfloat16
    x2 = np.asarray(x, dtype=np.float32).reshape(T, D)
    # xh[tt, kg, p, kc, n] = x2[tt*NT + n, kg*512 + kc*128 + p]
    xh = np.ascontiguousarray(
        x2.reshape(TT, NT, KG, 4, P).transpose(0, 2, 4, 3, 1)).astype(bf16)
    fc = np.asarray(freqs_cos, dtype=np.float32)
    fs = np.asarray(freqs_sin, dtype=np.float32)
    cc = np.repeat(fc.T, 2, axis=0)                         # [128, S]
    ss = np.repeat(fs.T, 2, axis=0)
    sgn = np.ones((P, 1), np.float32)
    sgn[0::2, 0] = -1.0
    cosE = np.ascontiguousarray(np.tile(cc, (1, B)).astype(np.float32))
    sinE = np.ascontiguousarray(np.tile(ss * sgn, (1, B)).astype(np.float32))
    perm_np = np.zeros((P, P), np.float32)
    for i in range(P):
        perm_np[i, i ^ 1] = 1.0
    perm_np = perm_np.astype(bf16)
    ident_np = np.eye(P, dtype=np.float32).astype(bf16)
    ones_np = np.ones((P, P), np.float32).astype(bf16)
    mk = np.zeros((P, NT // P, NT), np.float32)
    for r in range(NT // P):
        for p in range(P):
            mk[p, r, :] = np.where(np.arange(NT) >= 128 * r + p, 1.0, 0.0)
    mk = mk.astype(bf16)

    wq_f = np.asarray(wq, dtype=np.float32)
    wk_f = np.asarray(wk, dtype=np.float32)
    wv_f = np.asarray(wv, dtype=np.float32)
    wo_f = np.asarray(wo, dtype=np.float32)
    in_maps = []
    for g in range(8):
        wq_g = wq_f[:, g * NQ * HD:(g + 1) * NQ * HD]       # [D, 512]
        wk_g = wk_f[:, g * HD:(g + 1) * HD]                 # [D, 128]
        wv_g = wv_f[:, g * HD:(g + 1) * HD]
        wo_g = wo_f[g * NQ * HD:(g + 1) * NQ * HD, :]       # [512, D]
        # [P, DC, M] with element [p, c, m] = w[c*128 + p, m]
        wqh_np = np.ascontiguousarray(
            wq_g.reshape(DC, P, NQ * HD).transpose(1, 0, 2)).astype(bf16)
        wkh_np = np.ascontiguousarray(
            wk_g.reshape(DC, P, HD).transpose(1, 0, 2)).astype(bf16)
        wvh_np = np.ascontiguousarray(
            wv_g.reshape(DC, P, HD).transpose(1, 0, 2)).astype(bf16)
        # woh[p, j, dc, n] = wo_g[dc*128 + p, j*512 + n]
        woh_np = np.ascontiguousarray(
            wo_g.reshape(NQ, P, TT, NT).transpose(1, 2, 0, 3)).astype(bf16)
        in_maps.append({
            "xh": xh, "wqh": wqh_np, "wkh": wkh_np, "wvh": wvh_np,
            "woh": woh_np,
            "cosE": cosE, "sinE": sinE, "perm": perm_np, "ident": ident_np,
            "ones": ones_np, "maskneg": mk,
        })
    return in_maps


def combine_outputs(results):
    """Sum per-core partial^T and transpose back to [B, S, D]."""
    acc = results[0]["outT"].astype(np.float32)
    for r in results[1:]:
        acc += r["outT"].astype(np.float32)
    return np.ascontiguousarray(acc.T).reshape(B, S, D).astype(np.float32)


_NC = None


def kernel(x, wq, wk, wv, wo, freqs_cos, freqs_sin):
    """Full-input entry point: shards across 8 cores, runs, gathers."""
    global _NC
    from concourse.bass_utils import run_bass_kernel_spmd
    if _NC is None:
        _NC = build()
    in_maps = host_inputs(x, wq, wk, wv, wo, freqs_cos, freqs_sin)
    res = run_bass_kernel_spmd(_NC, in_maps, core_ids=list(range(8)),
                               trace=False)
    return combine_outputs(res.results)


# revision 14
# speedup vs baseline: 1.1152x; 1.0007x over previous
"""Self-contained Trainium2 Bass kernel for GQA causal self-attention.

Problem: x[2,2048,4096] @ wq/wk/wv (32 q-heads, 8 kv-heads, head_dim 128),
rope (precomputed freqs), causal softmax, GQA attention, wo projection.

Sharding: tensor-parallel across heads over 8 NeuronCores -- core g gets
kv-head g and q-heads 4g..4g+3 (wq/wk/wv column-sharded, wo row-sharded).
Each core computes a partial output projection; the host sums the 8
partials and transposes back (wo is row-parallel so partials just add).

v2: all tensors bf16 (halves DMA + enables FWL weight loads), Q^T and
attention outputs SBUF-resident (no DRAM spills), output projection fused
per 512-token block (software-pipelined one block behind attention), and
softmax denominators accumulated on the vector engine (bf16 adds) with
two small PE matmuls per (block, head) instead of one per k-chunk.
"""
import numpy as np
import ml_dtypes
import concourse.bacc as bacc
import concourse.mybir as mybir
import concourse.tile as tile

F32 = mybir.dt.float32
BF16 = mybir.dt.bfloat16
AF = mybir.ActivationFunctionType
OP = mybir.AluOpType

P = 128
B, S, D = 2, 2048, 4096
T = B * S            # 4096 tokens
HD = 128             # head dim
NQ = 4               # q heads per core
DC = D // P          # 32 contraction chunks
NT = 512             # free-dim tile
TT = T // NT         # 8 token tiles
SQT = S // NT        # 4 s_q tiles per batch
KG = 8               # k-chunk groups (4 chunks each) in phase 1
SCALE = 1.0 / float(np.sqrt(HD))


def build():
    nc = bacc.Bacc("TRN2", target_bir_lowering=False)
    # pre-shuffled inputs (see host_inputs)
    xh = nc.dram_tensor("xh", [TT, KG, P, 4, NT], BF16, kind="ExternalInput")
    wqh = nc.dram_tensor("wqh", [P, DC, NQ * HD], BF16, kind="ExternalInput")
    wkh = nc.dram_tensor("wkh", [P, DC, HD], BF16, kind="ExternalInput")
    wvh = nc.dram_tensor("wvh", [P, DC, HD], BF16, kind="ExternalInput")
    woh = nc.dram_tensor("woh", [P, TT, NQ, NT], BF16, kind="ExternalInput")
    cosE = nc.dram_tensor("cosE", [P, T], F32, kind="ExternalInput")
    sinE = nc.dram_tensor("sinE", [P, T], F32, kind="ExternalInput")
    perm = nc.dram_tensor("perm", [P, P], BF16, kind="ExternalInput")
    ident = nc.dram_tensor("ident", [P, P], BF16, kind="ExternalInput")
    ones = nc.dram_tensor("ones", [P, P], BF16, kind="ExternalInput")
    maskneg = nc.dram_tensor("maskneg", [P, NT // P, NT], BF16,
                             kind="ExternalInput")
    outT = nc.dram_tensor("outT", [D, T], BF16, kind="ExternalOutput")

    with tile.TileContext(nc) as tc:
        with (
            tc.tile_pool(name="res", bufs=1) as res,
            tc.tile_pool(name="outp", bufs=8) as outp,
        ):
            kT_res = res.tile([P, T], BF16)           # roped K^T, resident
            v_res = res.tile([P, DC, HD], BF16)       # V natural, resident
            qT_res = res.tile([P, NQ, T], BF16)       # roped Q^T, resident
            wo_sb = res.tile([P, TT, NQ, NT], BF16)   # wo, resident
            ones_sb = res.tile([P, P], BF16)
            nc.scalar.dma_start(ones_sb[:], ones[:, :])
            mask_sb = res.tile([P, NT // P, NT], BF16)
            nc.scalar.dma_start(mask_sb[:], maskneg[:, :, :])

            # ---------------- phase 1: projections + rope ----------------
            with (
                tc.tile_pool(name="c1", bufs=1) as c1,
                tc.tile_pool(name="xs", bufs=3) as xs,
                tc.tile_pool(name="epi", bufs=2) as epi,
                tc.tile_pool(name="ps_acc", bufs=7, space="PSUM") as ps_acc,
                tc.tile_pool(name="ps_misc", bufs=1, space="PSUM") as ps_misc,
            ):
                perm_sb = c1.tile([P, P], BF16)
                nc.scalar.dma_start(perm_sb[:], perm[:, :])
                # weights interleaved by k-group so tt0 matmuls start early
                wq_sb = c1.tile([P, DC, NQ * HD], BF16)
                wk_sb = c1.tile([P, DC, HD], BF16)
                wv_sb = c1.tile([P, DC, HD], BF16)

                for tt in range(TT):
                    tsl = slice(tt * NT, (tt + 1) * NT)
                    if tt == 2:
                        # deferred past the ramp so it doesn't steal HBM
                        # bandwidth from the first x/weight tiles
                        nc.scalar.dma_start(wo_sb[:], woh[:, :, :, :])
                    cos_t = xs.tile([P, NT], F32, tag="cos", bufs=2)
                    sin_t = xs.tile([P, NT], F32, tag="sin", bufs=2)
                    nc.scalar.dma_start(cos_t[:], cosE[:, tsl])
                    nc.scalar.dma_start(sin_t[:], sinE[:, tsl])

                    accs = [ps_acc.tile([P, NT], F32, tag="acc",
                                        name=f"acc{tt}_{oc}")
                            for oc in range(6)]
                    v_ps = accs[5]
                    for kg in range(KG):
                        xt = xs.tile([P, 4, NT], BF16, tag="xt", bufs=3)
                        nc.sync.dma_start(xt[:], xh[tt, kg, :, :, :])
                        if tt == 0:
                            ksl = slice(kg * 4, (kg + 1) * 4)
                            nc.sync.dma_start(wq_sb[:, ksl, :], wqh[:, ksl, :])
                            nc.sync.dma_start(wk_sb[:, ksl, :], wkh[:, ksl, :])
                            nc.sync.dma_start(wv_sb[:, ksl, :], wvh[:, ksl, :])
                        for kc in range(4):
                            k = kg * 4 + kc
                            for oc in range(6):
                                if oc < 4:
                                    lhsT = wq_sb[:, k, oc * P:(oc + 1) * P]
                                elif oc == 4:
                                    lhsT = wk_sb[:, k, :]
                                else:
                                    lhsT = wv_sb[:, k, :]
                                nc.tensor.matmul(accs[oc][:], lhsT=lhsT,
                                                 rhs=xt[:, kc, :],
                                                 start=(k == 0),
                                                 stop=(k == DC - 1))

                    # rope epilogue for Q (oc 0..3) and K (oc 4)
                    for oc in range(5):
                        acc = accs[oc]
                        sb_r = epi.tile([P, NT], BF16, tag="sbr")
                        nc.scalar.copy(sb_r[:], acc[:])
                        sw = ps_misc.tile([P, NT], F32, tag="misc", name="sw")
                        nc.tensor.matmul(sw[:], lhsT=perm_sb[:], rhs=sb_r[:],
                                         start=True, stop=True)
                        t1 = epi.tile([P, NT], F32, tag="t1")
                        nc.vector.tensor_tensor(t1[:], acc[:], cos_t[:],
                                                op=OP.mult)
                        t2 = epi.tile([P, NT], F32, tag="t2")
                        nc.vector.tensor_tensor(t2[:], sw[:], sin_t[:],
                                                op=OP.mult)
                        if oc < 4:
                            nc.vector.tensor_tensor(qT_res[:, oc, tsl],
                                                    t1[:], t2[:], op=OP.add)
                        else:
                            nc.vector.tensor_tensor(kT_res[:, tsl],
                                                    t1[:], t2[:], op=OP.add)

                    # V epilogue: V^T chunks -> DMA xbar transpose -> V
                    vsb = epi.tile([P, NT], BF16, tag="sbr")
                    nc.scalar.copy(vsb[:], v_ps[:])
                    for c in range(NT // P):
                        nc.sync.dma_start(v_res[:, 4 * tt + c, :],
                                          vsb[:, c * P:(c + 1) * P],
                                          transpose=True)

            # ---------- phase 2+3 fused: attention + output proj ----------
            with (
                tc.tile_pool(name="pts", bufs=3) as pts,
                tc.tile_pool(name="ep2", bufs=2) as ep2,
                tc.tile_pool(name="ats", bufs=2) as ats,
                tc.tile_pool(name="ps_st", bufs=2, space="PSUM") as ps_st,
                tc.tile_pool(name="ps_att", bufs=2, space="PSUM") as ps_att,
                tc.tile_pool(name="ps_o", bufs=2, space="PSUM") as ps_o,
            ):
                def emit_sweep(jt, at):
                    jsl = slice(jt * NT, (jt + 1) * NT)
                    for oc in range(DC):
                        o_ps = ps_o.tile([P, NT], F32, tag="o")
                        wj, wn = oc // 4, (oc % 4) * P
                        for dc in range(NQ):
                            nc.tensor.matmul(o_ps[:],
                                             lhsT=wo_sb[:, wj, dc, wn:wn + P],
                                             rhs=at[:, dc, :],
                                             start=(dc == 0),
                                             stop=(dc == NQ - 1))
                        osb = outp.tile([P, NT], BF16, tag="ot")
                        if oc % 2 == 0:
                            nc.scalar.copy(osb[:], o_ps[:])
                        else:
                            nc.vector.tensor_copy(osb[:], o_ps[:])
                        nc.sync.dma_start(outT[oc * P:(oc + 1) * P, jsl],
                                          osb[:])

                prev = None
                # jq descending: the first block (no prior sweep to hide
                # exp latency behind) gets the deepest matmul pipeline
                for b in range(B):
                    for jq in sorted(range(SQT), reverse=(b == 0)):
                        nk = 4 * (jq + 1)
                        qsl = slice(b * S + jq * NT, b * S + (jq + 1) * NT)
                        at = ats.tile([P, NQ, NT], BF16, tag="at")
                        for h in range(NQ):
                            att_ps = ps_att.tile([P, NT], F32, tag="att")
                            acc = pts.tile([P, 2, NT], BF16, tag="acc",
                                           bufs=2)
                            for ip in range(nk // 2):
                                st = ps_st.tile([P, 2, NT], F32, tag="st")
                                for half in range(2):
                                    ik = 2 * ip + half
                                    nc.tensor.matmul(
                                        st[:, half, :],
                                        lhsT=kT_res[:, b * S + ik * P:
                                                    b * S + (ik + 1) * P],
                                        rhs=qT_res[:, h, qsl],
                                        start=True, stop=True)
                                dst = acc if ip == 0 else pts.tile(
                                    [P, 2, NT], BF16, tag="pt", bufs=3)
                                r = 2 * ip - 4 * jq
                                if r >= 0:
                                    pr = pts.tile([P, 2, NT], BF16,
                                                  tag="pr", bufs=2)
                                    nc.scalar.activation(pr[:, :, :],
                                                         st[:, :, :],
                                                         AF.Exp, scale=SCALE)
                                    nc.vector.tensor_tensor(
                                        dst[:, :, :], pr[:, :, :],
                                        mask_sb[:, r:r + 2, :], op=OP.mult)
                                else:
                                    nc.scalar.activation(dst[:, :, :],
                                                         st[:, :, :],
                                                         AF.Exp, scale=SCALE)
                                if ip > 0:
                                    nc.vector.tensor_tensor(
                                        acc[:, :, :], acc[:, :, :],
                                        dst[:, :, :], op=OP.add)
                                for half in range(2):
                                    ik = 2 * ip + half
                                    nc.tensor.matmul(
                                        att_ps[:],
                                        lhsT=v_res[:, 16 * b + ik, :],
                                        rhs=dst[:, half, :],
                                        start=(ik == 0), stop=(ik == nk - 1))
                            den_ps = ps_o.tile([P, NT], F32, tag="o")
                            for half in range(2):
                                nc.tensor.matmul(den_ps[:], lhsT=ones_sb[:],
                                                 rhs=acc[:, half, :],
                                                 start=(half == 0),
                                                 stop=(half == 1))
                            rc = ep2.tile([P, NT], F32, tag="rc")
                            nc.vector.reciprocal_approx_fast(rc[:], den_ps[:])
                            nc.vector.tensor_tensor(at[:, h, :], att_ps[:],
                                                    rc[:], op=OP.mult)
                        if prev is not None:
                            emit_sweep(*prev)
                        prev = (b * SQT + jq, at)
                emit_sweep(*prev)

    nc.compile()
    return nc


def host_inputs(x, wq, wk, wv, wo, freqs_cos, freqs_sin):
    """Build the 8 per-core input maps from full inputs (pre-shuffled)."""
    bf16 = ml_dtypes.bfloat16
    x2 = np.asarray(x, dtype=np.float32).reshape(T, D)
    # xh[tt, kg, p, kc, n] = x2[tt*NT + n, kg*512 + kc*128 + p]
    xh = np.ascontiguousarray(
        x2.reshape(TT, NT, KG, 4, P).transpose(0, 2, 4, 3, 1)).astype(bf16)
    fc = np.asarray(freqs_cos, dtype=np.float32)
    fs = np.asarray(freqs_sin, dtype=np.float32)
    cc = np.repeat(fc.T, 2, axis=0)                         # [128, S]
    ss = np.repeat(fs.T, 2, axis=0)
    sgn = np.ones((P, 1), np.float32)
    sgn[0::2, 0] = -1.0
    cosE = np.ascontiguousarray(np.tile(cc, (1, B)).astype(np.float32))
    sinE = np.ascontiguousarray(np.tile(ss * sgn, (1, B)).astype(np.float32))
    perm_np = np.zeros((P, P), np.float32)
    for i in range(P):
        perm_np[i, i ^ 1] = 1.0
    perm_np = perm_np.astype(bf16)
    ident_np = np.eye(P, dtype=np.float32).astype(bf16)
    ones_np = np.ones((P, P), np.float32).astype(bf16)
    mk = np.zeros((P, NT // P, NT), np.float32)
    for r in range(NT // P):
        for p in range(P):
            mk[p, r, :] = np.where(np.arange(NT) >= 128 * r + p, 1.0, 0.0)
    mk = mk.astype(bf16)

    wq_f = np.asarray(wq, dtype=np.float32)
    wk_f = np.asarray(wk, dtype=np.float32)
    wv_f = np.asarray(wv, dtype=np.float32)
    wo_f = np.asarray(wo, dtype=np.float32)
    in_maps = []
    for g in range(8):
        wq_g = wq_f[:, g * NQ * HD:(g + 1) * NQ * HD]       # [D, 512]
        wk_g = wk_f[:, g * HD:(g + 1) * HD]                 # [D, 128]
        wv_g = wv_f[:, g * HD:(g + 1) * HD]
        wo_g = wo_f[g * NQ * HD:(g + 1) * NQ * HD, :]       # [512, D]
        # [P, DC, M] with element [p, c, m] = w[c*128 + p, m]
        wqh_np = np.ascontiguousarray(
            wq_g.reshape(DC, P, NQ * HD).transpose(1, 0, 2)).astype(bf16)
        wkh_np = np.ascontiguousarray(
            wk_g.reshape(DC, P, HD).transpose(1, 0, 2)).astype(bf16)
        wvh_np = np.ascontiguousarray(
            wv_g.reshape(DC, P, HD).transpose(1, 0, 2)).astype(bf16)
        # woh[p, j, dc, n] = wo_g[dc*128 + p, j*512 + n]
        woh_np = np.ascontiguousarray(
            wo_g.reshape(NQ, P, TT, NT).transpose(1, 2, 0, 3)).astype(bf16)
        in_maps.append({
            "xh": xh, "wqh": wqh_np, "wkh": wkh_np, "wvh": wvh_np,
            "woh": woh_np,
            "cosE": cosE, "sinE": sinE, "perm": perm_np, "ident": ident_np,
            "ones": ones_np, "maskneg": mk,
        })
    return in_maps


def combine_outputs(results):
    """Sum per-core partial^T and transpose back to [B, S, D]."""
    acc = results[0]["outT"].astype(np.float32)
    for r in results[1:]:
        acc += r["outT"].astype(np.float32)
    return np.ascontiguousarray(acc.T).reshape(B, S, D).astype(np.float32)


_NC = None


def kernel(x, wq, wk, wv, wo, freqs_cos, freqs_sin):
    """Full-input entry point: shards across 8 cores, runs, gathers."""
    global _NC
    from concourse.bass_utils import run_bass_kernel_spmd
    if _NC is None:
        _NC = build()
    in_maps = host_inputs(x, wq, wk, wv, wo, freqs_cos, freqs_sin)
    res = run_bass_kernel_spmd(_NC, in_maps, core_ids=list(range(8)),
                               trace=False)
    return combine_outputs(res.results)


# revision 16
# speedup vs baseline: 1.1168x; 1.0014x over previous
"""Self-contained Trainium2 Bass kernel for GQA causal self-attention.

Problem: x[2,2048,4096] @ wq/wk/wv (32 q-heads, 8 kv-heads, head_dim 128),
rope (precomputed freqs), causal softmax, GQA attention, wo projection.

Sharding: tensor-parallel across heads over 8 NeuronCores -- core g gets
kv-head g and q-heads 4g..4g+3 (wq/wk/wv column-sharded, wo row-sharded).
Each core computes a partial output projection; the host sums the 8
partials and transposes back (wo is row-parallel so partials just add).

v2: all tensors bf16 (halves DMA + enables FWL weight loads), Q^T and
attention outputs SBUF-resident (no DRAM spills), output projection fused
per 512-token block (software-pipelined one block behind attention), and
softmax denominators accumulated on the vector engine (bf16 adds) with
two small PE matmuls per (block, head) instead of one per k-chunk.
"""
import numpy as np
import ml_dtypes
import concourse.bacc as bacc
import concourse.mybir as mybir
import concourse.tile as tile

F32 = mybir.dt.float32
BF16 = mybir.dt.bfloat16
AF = mybir.ActivationFunctionType
OP = mybir.AluOpType

P = 128
B, S, D = 2, 2048, 4096
T = B * S            # 4096 tokens
HD = 128             # head dim
NQ = 4               # q heads per core
DC = D // P          # 32 contraction chunks
NT = 512             # free-dim tile
TT = T // NT         # 8 token tiles
SQT = S // NT        # 4 s_q tiles per batch
KG = 8               # k-chunk groups (4 chunks each) in phase 1
SCALE = 1.0 / float(np.sqrt(HD))


def build():
    nc = bacc.Bacc("TRN2", target_bir_lowering=False)
    # pre-shuffled inputs (see host_inputs)
    xh = nc.dram_tensor("xh", [TT, KG, P, 4, NT], BF16, kind="ExternalInput")
    wqh = nc.dram_tensor("wqh", [P, DC, NQ * HD], BF16, kind="ExternalInput")
    wkh = nc.dram_tensor("wkh", [P, DC, HD], BF16, kind="ExternalInput")
    wvh = nc.dram_tensor("wvh", [P, DC, HD], BF16, kind="ExternalInput")
    woh = nc.dram_tensor("woh", [P, TT, NQ, NT], BF16, kind="ExternalInput")
    cosE = nc.dram_tensor("cosE", [P, T], F32, kind="ExternalInput")
    sinE = nc.dram_tensor("sinE", [P, T], F32, kind="ExternalInput")
    perm = nc.dram_tensor("perm", [P, P], BF16, kind="ExternalInput")
    ident = nc.dram_tensor("ident", [P, P], BF16, kind="ExternalInput")
    ones = nc.dram_tensor("ones", [P, P], BF16, kind="ExternalInput")
    maskneg = nc.dram_tensor("maskneg", [P, NT // P, NT], BF16,
                             kind="ExternalInput")
    outT = nc.dram_tensor("outT", [D, T], BF16, kind="ExternalOutput")

    with tile.TileContext(nc) as tc:
        with (
            tc.tile_pool(name="res", bufs=1) as res,
            tc.tile_pool(name="outp", bufs=8) as outp,
        ):
            kT_res = res.tile([P, T], BF16)           # roped K^T, resident
            v_res = res.tile([P, DC, HD], BF16)       # V natural, resident
            qT_res = res.tile([P, NQ, T], BF16)       # roped Q^T, resident
            wo_sb = res.tile([P, TT, NQ, NT], BF16)   # wo, resident
            ones_sb = res.tile([P, P], BF16)
            nc.scalar.dma_start(ones_sb[:], ones[:, :])
            mask_sb = res.tile([P, NT // P, NT], BF16)
            nc.scalar.dma_start(mask_sb[:], maskneg[:, :, :])

            # ---------------- phase 1: projections + rope ----------------
            with (
                tc.tile_pool(name="c1", bufs=1) as c1,
                tc.tile_pool(name="xs", bufs=3) as xs,
                tc.tile_pool(name="epi", bufs=2) as epi,
                tc.tile_pool(name="ps_acc", bufs=7, space="PSUM") as ps_acc,
                tc.tile_pool(name="ps_misc", bufs=1, space="PSUM") as ps_misc,
            ):
                perm_sb = c1.tile([P, P], BF16)
                nc.scalar.dma_start(perm_sb[:], perm[:, :])
                # weights interleaved by k-group so tt0 matmuls start early
                wq_sb = c1.tile([P, DC, NQ * HD], BF16)
                wk_sb = c1.tile([P, DC, HD], BF16)
                wv_sb = c1.tile([P, DC, HD], BF16)

                # weights on the ACT HWDGE ring (separate FIFO from the x
                # stream on SP), batched coarser as the deadline recedes
                for ksl in (slice(0, 4), slice(4, 8), slice(8, 16),
                            slice(16, 32)):
                    nc.scalar.dma_start(wq_sb[:, ksl, :], wqh[:, ksl, :])
                    nc.scalar.dma_start(wk_sb[:, ksl, :], wkh[:, ksl, :])
                    nc.scalar.dma_start(wv_sb[:, ksl, :], wvh[:, ksl, :])

                for tt in range(TT):
                    tsl = slice(tt * NT, (tt + 1) * NT)
                    if tt == 2:
                        # deferred past the ramp so it doesn't steal HBM
                        # bandwidth from the first x/weight tiles
                        nc.scalar.dma_start(wo_sb[:], woh[:, :, :, :])
                    cos_t = xs.tile([P, NT], F32, tag="cos", bufs=2)
                    sin_t = xs.tile([P, NT], F32, tag="sin", bufs=2)
                    nc.scalar.dma_start(cos_t[:], cosE[:, tsl])
                    nc.scalar.dma_start(sin_t[:], sinE[:, tsl])

                    accs = [ps_acc.tile([P, NT], F32, tag="acc",
                                        name=f"acc{tt}_{oc}")
                            for oc in range(6)]
                    v_ps = accs[5]
                    for kg in range(KG):
                        xt = xs.tile([P, 4, NT], BF16, tag="xt", bufs=3)
                        nc.sync.dma_start(xt[:], xh[tt, kg, :, :, :])
                        for kc in range(4):
                            k = kg * 4 + kc
                            for oc in range(6):
                                if oc < 4:
                                    lhsT = wq_sb[:, k, oc * P:(oc + 1) * P]
                                elif oc == 4:
                                    lhsT = wk_sb[:, k, :]
                                else:
                                    lhsT = wv_sb[:, k, :]
                                nc.tensor.matmul(accs[oc][:], lhsT=lhsT,
                                                 rhs=xt[:, kc, :],
                                                 start=(k == 0),
                                                 stop=(k == DC - 1))

                    # rope epilogue for Q (oc 0..3) and K (oc 4)
                    for oc in range(5):
                        acc = accs[oc]
                        sb_r = epi.tile([P, NT], BF16, tag="sbr")
                        nc.scalar.copy(sb_r[:], acc[:])
                        sw = ps_misc.tile([P, NT], F32, tag="misc", name="sw")
                        nc.tensor.matmul(sw[:], lhsT=perm_sb[:], rhs=sb_r[:],
                                         start=True, stop=True)
                        t1 = epi.tile([P, NT], F32, tag="t1")
                        nc.vector.tensor_tensor(t1[:], acc[:], cos_t[:],
                                                op=OP.mult)
                        t2 = epi.tile([P, NT], F32, tag="t2")
                        nc.vector.tensor_tensor(t2[:], sw[:], sin_t[:],
                                                op=OP.mult)
                        if oc < 4:
                            nc.vector.tensor_tensor(qT_res[:, oc, tsl],
                                                    t1[:], t2[:], op=OP.add)
                        else:
                            nc.vector.tensor_tensor(kT_res[:, tsl],
                                                    t1[:], t2[:], op=OP.add)

                    # V epilogue: V^T chunks -> DMA xbar transpose -> V
                    vsb = epi.tile([P, NT], BF16, tag="sbr")
                    nc.scalar.copy(vsb[:], v_ps[:])
                    for c in range(NT // P):
                        nc.sync.dma_start(v_res[:, 4 * tt + c, :],
                                          vsb[:, c * P:(c + 1) * P],
                                          transpose=True)

            # ---------- phase 2+3 fused: attention + output proj ----------
            with (
                tc.tile_pool(name="pts", bufs=3) as pts,
                tc.tile_pool(name="ep2", bufs=2) as ep2,
                tc.tile_pool(name="ats", bufs=2) as ats,
                tc.tile_pool(name="ps_st", bufs=2, space="PSUM") as ps_st,
                tc.tile_pool(name="ps_att", bufs=2, space="PSUM") as ps_att,
                tc.tile_pool(name="ps_o", bufs=2, space="PSUM") as ps_o,
            ):
                def emit_sweep(jt, at):
                    jsl = slice(jt * NT, (jt + 1) * NT)
                    for oc in range(DC):
                        o_ps = ps_o.tile([P, NT], F32, tag="o")
                        wj, wn = oc // 4, (oc % 4) * P
                        for dc in range(NQ):
                            nc.tensor.matmul(o_ps[:],
                                             lhsT=wo_sb[:, wj, dc, wn:wn + P],
                                             rhs=at[:, dc, :],
                                             start=(dc == 0),
                                             stop=(dc == NQ - 1))
                        osb = outp.tile([P, NT], BF16, tag="ot")
                        if oc % 2 == 0:
                            nc.scalar.copy(osb[:], o_ps[:])
                        else:
                            nc.vector.tensor_copy(osb[:], o_ps[:])
                        nc.sync.dma_start(outT[oc * P:(oc + 1) * P, jsl],
                                          osb[:])

                prev = None
                # jq descending: the first block (no prior sweep to hide
                # exp latency behind) gets the deepest matmul pipeline
                for b in range(B):
                    for jq in sorted(range(SQT), reverse=(b == 0)):
                        nk = 4 * (jq + 1)
                        qsl = slice(b * S + jq * NT, b * S + (jq + 1) * NT)
                        at = ats.tile([P, NQ, NT], BF16, tag="at")
                        for h in range(NQ):
                            att_ps = ps_att.tile([P, NT], F32, tag="att")
                            acc = pts.tile([P, 2, NT], BF16, tag="acc",
                                           bufs=2)
                            for ip in range(nk // 2):
                                st = ps_st.tile([P, 2, NT], F32, tag="st")
                                for half in range(2):
                                    ik = 2 * ip + half
                                    nc.tensor.matmul(
                                        st[:, half, :],
                                        lhsT=kT_res[:, b * S + ik * P:
                                                    b * S + (ik + 1) * P],
                                        rhs=qT_res[:, h, qsl],
                                        start=True, stop=True)
                                dst = acc if ip == 0 else pts.tile(
                                    [P, 2, NT], BF16, tag="pt", bufs=3)
                                r = 2 * ip - 4 * jq
                                if r >= 0:
                                    pr = pts.tile([P, 2, NT], BF16,
                                                  tag="pr", bufs=2)
                                    nc.scalar.activation(pr[:, :, :],
                                                         st[:, :, :],
                                                         AF.Exp, scale=SCALE)
                                    nc.vector.tensor_tensor(
                                        dst[:, :, :], pr[:, :, :],
                                        mask_sb[:, r:r + 2, :], op=OP.mult)
                                else:
                                    nc.scalar.activation(dst[:, :, :],
                                                         st[:, :, :],
                                                         AF.Exp, scale=SCALE)
                                if ip > 0:
                                    nc.vector.tensor_tensor(
                                        acc[:, :, :], acc[:, :, :],
                                        dst[:, :, :], op=OP.add)
                                for half in range(2):
                                    ik = 2 * ip + half
                                    nc.tensor.matmul(
                                        att_ps[:],
                                        lhsT=v_res[:, 16 * b + ik, :],
                                        rhs=dst[:, half, :],
                                        start=(ik == 0), stop=(ik == nk - 1))
                            den_ps = ps_o.tile([P, NT], F32, tag="o")
                            for half in range(2):
                                nc.tensor.matmul(den_ps[:], lhsT=ones_sb[:],
                                                 rhs=acc[:, half, :],
                                                 start=(half == 0),
                                                 stop=(half == 1))
                            rc = ep2.tile([P, NT], F32, tag="rc")
                            nc.vector.reciprocal_approx_fast(rc[:], den_ps[:])
                            nc.vector.tensor_tensor(at[:, h, :], att_ps[:],
                                                    rc[:], op=OP.mult)
                        if prev is not None:
                            emit_sweep(*prev)
                        prev = (b * SQT + jq, at)
                emit_sweep(*prev)

    nc.compile()
    return nc


def host_inputs(x, wq, wk, wv, wo, freqs_cos, freqs_sin):
    """Build the 8 per-core input maps from full inputs (pre-shuffled)."""
    bf16 = ml_dtypes.bfloat16
    x2 = np.asarray(x, dtype=np.float32).reshape(T, D)
    # xh[tt, kg, p, kc, n] = x2[tt*NT + n, kg*512 + kc*128 + p]
    xh = np.ascontiguousarray(
        x2.reshape(TT, NT, KG, 4, P).transpose(0, 2, 4, 3, 1)).astype(bf16)
    fc = np.asarray(freqs_cos, dtype=np.float32)
    fs = np.asarray(freqs_sin, dtype=np.float32)
    cc = np.repeat(fc.T, 2, axis=0)                         # [128, S]
    ss = np.repeat(fs.T, 2, axis=0)
    sgn = np.ones((P, 1), np.float32)
    sgn[0::2, 0] = -1.0
    cosE = np.ascontiguousarray(np.tile(cc, (1, B)).astype(np.float32))
    sinE = np.ascontiguousarray(np.tile(ss * sgn, (1, B)).astype(np.float32))
    perm_np = np.zeros((P, P), np.float32)
    for i in range(P):
        perm_np[i, i ^ 1] = 1.0
    perm_np = perm_np.astype(bf16)
    ident_np = np.eye(P, dtype=np.float32).astype(bf16)
    ones_np = np.ones((P, P), np.float32).astype(bf16)
    mk = np.zeros((P, NT // P, NT), np.float32)
    for r in range(NT // P):
        for p in range(P):
            mk[p, r, :] = np.where(np.arange(NT) >= 128 * r + p, 1.0, 0.0)
    mk = mk.astype(bf16)

    wq_f = np.asarray(wq, dtype=np.float32)
    wk_f = np.asarray(wk, dtype=np.float32)
    wv_f = np.asarray(wv, dtype=np.float32)
    wo_f = np.asarray(wo, dtype=np.float32)
    in_maps = []
    for g in range(8):
        wq_g = wq_f[:, g * NQ * HD:(g + 1) * NQ * HD]       # [D, 512]
        wk_g = wk_f[:, g * HD:(g + 1) * HD]                 # [D, 128]
        wv_g = wv_f[:, g * HD:(g + 1) * HD]
        wo_g = wo_f[g * NQ * HD:(g + 1) * NQ * HD, :]       # [512, D]
        # [P, DC, M] with element [p, c, m] = w[c*128 + p, m]
        wqh_np = np.ascontiguousarray(
            wq_g.reshape(DC, P, NQ * HD).transpose(1, 0, 2)).astype(bf16)
        wkh_np = np.ascontiguousarray(
            wk_g.reshape(DC, P, HD).transpose(1, 0, 2)).astype(bf16)
        wvh_np = np.ascontiguousarray(
            wv_g.reshape(DC, P, HD).transpose(1, 0, 2)).astype(bf16)
        # woh[p, j, dc, n] = wo_g[dc*128 + p, j*512 + n]
        woh_np = np.ascontiguousarray(
            wo_g.reshape(NQ, P, TT, NT).transpose(1, 2, 0, 3)).astype(bf16)
        in_maps.append({
            "xh": xh, "wqh": wqh_np, "wkh": wkh_np, "wvh": wvh_np,
            "woh": woh_np,
            "cosE": cosE, "sinE": sinE, "perm": perm_np, "ident": ident_np,
            "ones": ones_np, "maskneg": mk,
        })
    return in_maps


def combine_outputs(results):
    """Sum per-core partial^T and transpose back to [B, S, D]."""
    acc = results[0]["outT"].astype(np.float32)
    for r in results[1:]:
        acc += r["outT"].astype(np.float32)
    return np.ascontiguousarray(acc.T).reshape(B, S, D).astype(np.float32)


_NC = None


def kernel(x, wq, wk, wv, wo, freqs_cos, freqs_sin):
    """Full-input entry point: shards across 8 cores, runs, gathers."""
    global _NC
    from concourse.bass_utils import run_bass_kernel_spmd
    if _NC is None:
        _NC = build()
    in_maps = host_inputs(x, wq, wk, wv, wo, freqs_cos, freqs_sin)
    res = run_bass_kernel_spmd(_NC, in_maps, core_ids=list(range(8)),
                               trace=False)
    return combine_outputs(res.results)


# revision 17
# speedup vs baseline: 1.1269x; 1.0090x over previous
"""Self-contained Trainium2 Bass kernel for GQA causal self-attention.

Problem: x[2,2048,4096] @ wq/wk/wv (32 q-heads, 8 kv-heads, head_dim 128),
rope (precomputed freqs), causal softmax, GQA attention, wo projection.

Sharding: tensor-parallel across heads over 8 NeuronCores -- core g gets
kv-head g and q-heads 4g..4g+3 (wq/wk/wv column-sharded, wo row-sharded).
Each core computes a partial output projection; the host sums the 8
partials and transposes back (wo is row-parallel so partials just add).

v2: all tensors bf16 (halves DMA + enables FWL weight loads), Q^T and
attention outputs SBUF-resident (no DRAM spills), output projection fused
per 512-token block (software-pipelined one block behind attention), and
softmax denominators accumulated on the vector engine (bf16 adds) with
two small PE matmuls per (block, head) instead of one per k-chunk.
"""
import numpy as np
import ml_dtypes
import concourse.bacc as bacc
import concourse.mybir as mybir
import concourse.tile as tile

F32 = mybir.dt.float32
BF16 = mybir.dt.bfloat16
AF = mybir.ActivationFunctionType
OP = mybir.AluOpType

P = 128
B, S, D = 2, 2048, 4096
T = B * S            # 4096 tokens
HD = 128             # head dim
NQ = 4               # q heads per core
DC = D // P          # 32 contraction chunks
NT = 512             # free-dim tile
TT = T // NT         # 8 token tiles
SQT = S // NT        # 4 s_q tiles per batch
KG = 8               # k-chunk groups (4 chunks each) in phase 1
SCALE = 1.0 / float(np.sqrt(HD))


def build():
    nc = bacc.Bacc("TRN2", target_bir_lowering=False)
    # pre-shuffled inputs (see host_inputs)
    xh = nc.dram_tensor("xh", [TT, KG, P, 4, NT], BF16, kind="ExternalInput")
    wqh = nc.dram_tensor("wqh", [P, DC, NQ * HD], BF16, kind="ExternalInput")
    wkh = nc.dram_tensor("wkh", [P, DC, HD], BF16, kind="ExternalInput")
    wvh = nc.dram_tensor("wvh", [P, DC, HD], BF16, kind="ExternalInput")
    woh = nc.dram_tensor("woh", [P, TT, NQ, NT], BF16, kind="ExternalInput")
    cosE = nc.dram_tensor("cosE", [P, T], F32, kind="ExternalInput")
    sinE = nc.dram_tensor("sinE", [P, T], F32, kind="ExternalInput")
    perm = nc.dram_tensor("perm", [P, P], BF16, kind="ExternalInput")
    ident = nc.dram_tensor("ident", [P, P], BF16, kind="ExternalInput")
    ones = nc.dram_tensor("ones", [P, P], BF16, kind="ExternalInput")
    maskneg = nc.dram_tensor("maskneg", [P, NT // P, NT], BF16,
                             kind="ExternalInput")
    outT = nc.dram_tensor("outT", [D, T], BF16, kind="ExternalOutput")

    with tile.TileContext(nc) as tc:
        with (
            tc.tile_pool(name="res", bufs=1) as res,
            tc.tile_pool(name="outp", bufs=8) as outp,
        ):
            kT_res = res.tile([P, T], BF16)           # roped K^T, resident
            v_res = res.tile([P, DC, HD], BF16)       # V natural, resident
            qT_res = res.tile([P, NQ, T], BF16)       # roped Q^T, resident
            wo_sb = res.tile([P, TT, NQ, NT], BF16)   # wo, resident
            ones_sb = res.tile([P, P], BF16)
            nc.scalar.dma_start(ones_sb[:], ones[:, :])
            mask_sb = res.tile([P, NT // P, NT], BF16)
            nc.scalar.dma_start(mask_sb[:], maskneg[:, :, :])

            # ---------------- phase 1: projections + rope ----------------
            with (
                tc.tile_pool(name="c1", bufs=1) as c1,
                tc.tile_pool(name="xs", bufs=3) as xs,
                tc.tile_pool(name="epi", bufs=2) as epi,
                tc.tile_pool(name="ps_acc", bufs=7, space="PSUM") as ps_acc,
                tc.tile_pool(name="ps_misc", bufs=1, space="PSUM") as ps_misc,
            ):
                perm_sb = c1.tile([P, P], BF16)
                nc.scalar.dma_start(perm_sb[:], perm[:, :])
                # weights interleaved by k-group so tt0 matmuls start early
                wq_sb = c1.tile([P, DC, NQ * HD], BF16)
                wk_sb = c1.tile([P, DC, HD], BF16)
                wv_sb = c1.tile([P, DC, HD], BF16)

                # weights on the ACT HWDGE ring (separate FIFO from the x
                # stream on SP), batched coarser as the deadline recedes
                for ksl in (slice(0, 4), slice(4, 8), slice(8, 16),
                            slice(16, 32)):
                    nc.scalar.dma_start(wq_sb[:, ksl, :], wqh[:, ksl, :])
                    nc.scalar.dma_start(wk_sb[:, ksl, :], wkh[:, ksl, :])
                    nc.scalar.dma_start(wv_sb[:, ksl, :], wvh[:, ksl, :])

                for tt in range(TT):
                    tsl = slice(tt * NT, (tt + 1) * NT)
                    if tt == 2:
                        # deferred past the ramp so it doesn't steal HBM
                        # bandwidth from the first x/weight tiles
                        nc.scalar.dma_start(wo_sb[:], woh[:, :, :, :])
                    cos_t = xs.tile([P, NT], F32, tag="cos", bufs=2)
                    sin_t = xs.tile([P, NT], F32, tag="sin", bufs=2)
                    nc.scalar.dma_start(cos_t[:], cosE[:, tsl])
                    nc.scalar.dma_start(sin_t[:], sinE[:, tsl])

                    accs = [ps_acc.tile([P, NT], F32, tag="acc",
                                        name=f"acc{tt}_{oc}")
                            for oc in range(6)]
                    v_ps = accs[5]
                    for kg in range(KG):
                        xt = xs.tile([P, 4, NT], BF16, tag="xt", bufs=3)
                        if tt == 0 and kg == 0:
                            # split so the first matmuls start on slice 0
                            # while the DMA engines are still ramping
                            for kc in range(4):
                                nc.sync.dma_start(xt[:, kc, :],
                                                  xh[tt, kg, :, kc, :])
                        else:
                            nc.sync.dma_start(xt[:], xh[tt, kg, :, :, :])
                        for kc in range(4):
                            k = kg * 4 + kc
                            for oc in range(6):
                                if oc < 4:
                                    lhsT = wq_sb[:, k, oc * P:(oc + 1) * P]
                                elif oc == 4:
                                    lhsT = wk_sb[:, k, :]
                                else:
                                    lhsT = wv_sb[:, k, :]
                                nc.tensor.matmul(accs[oc][:], lhsT=lhsT,
                                                 rhs=xt[:, kc, :],
                                                 start=(k == 0),
                                                 stop=(k == DC - 1))

                    # rope epilogue for Q (oc 0..3) and K (oc 4)
                    for oc in range(5):
                        acc = accs[oc]
                        sb_r = epi.tile([P, NT], BF16, tag="sbr")
                        nc.scalar.copy(sb_r[:], acc[:])
                        sw = ps_misc.tile([P, NT], F32, tag="misc", name="sw")
                        nc.tensor.matmul(sw[:], lhsT=perm_sb[:], rhs=sb_r[:],
                                         start=True, stop=True)
                        t1 = epi.tile([P, NT], F32, tag="t1")
                        nc.vector.tensor_tensor(t1[:], acc[:], cos_t[:],
                                                op=OP.mult)
                        t2 = epi.tile([P, NT], F32, tag="t2")
                        nc.vector.tensor_tensor(t2[:], sw[:], sin_t[:],
                                                op=OP.mult)
                        if oc < 4:
                            nc.vector.tensor_tensor(qT_res[:, oc, tsl],
                                                    t1[:], t2[:], op=OP.add)
                        else:
                            nc.vector.tensor_tensor(kT_res[:, tsl],
                                                    t1[:], t2[:], op=OP.add)

                    # V epilogue: V^T chunks -> DMA xbar transpose -> V
                    vsb = epi.tile([P, NT], BF16, tag="sbr")
                    nc.scalar.copy(vsb[:], v_ps[:])
                    for c in range(NT // P):
                        nc.sync.dma_start(v_res[:, 4 * tt + c, :],
                                          vsb[:, c * P:(c + 1) * P],
                                          transpose=True)

            # ---------- phase 2+3 fused: attention + output proj ----------
            with (
                tc.tile_pool(name="pts", bufs=3) as pts,
                tc.tile_pool(name="ep2", bufs=2) as ep2,
                tc.tile_pool(name="ats", bufs=2) as ats,
                tc.tile_pool(name="ps_st", bufs=2, space="PSUM") as ps_st,
                tc.tile_pool(name="ps_att", bufs=2, space="PSUM") as ps_att,
                tc.tile_pool(name="ps_o", bufs=2, space="PSUM") as ps_o,
            ):
                def emit_sweep(jt, at):
                    jsl = slice(jt * NT, (jt + 1) * NT)
                    for oc in range(DC):
                        o_ps = ps_o.tile([P, NT], F32, tag="o")
                        wj, wn = oc // 4, (oc % 4) * P
                        for dc in range(NQ):
                            nc.tensor.matmul(o_ps[:],
                                             lhsT=wo_sb[:, wj, dc, wn:wn + P],
                                             rhs=at[:, dc, :],
                                             start=(dc == 0),
                                             stop=(dc == NQ - 1))
                        osb = outp.tile([P, NT], BF16, tag="ot")
                        if oc % 2 == 0:
                            nc.scalar.copy(osb[:], o_ps[:])
                        else:
                            nc.vector.tensor_copy(osb[:], o_ps[:])
                        nc.sync.dma_start(outT[oc * P:(oc + 1) * P, jsl],
                                          osb[:])

                prev = None
                # jq descending: the first block (no prior sweep to hide
                # exp latency behind) gets the deepest matmul pipeline
                for b in range(B):
                    for jq in sorted(range(SQT), reverse=(b == 0)):
                        nk = 4 * (jq + 1)
                        qsl = slice(b * S + jq * NT, b * S + (jq + 1) * NT)
                        at = ats.tile([P, NQ, NT], BF16, tag="at")
                        for h in range(NQ):
                            att_ps = ps_att.tile([P, NT], F32, tag="att")
                            acc = pts.tile([P, 2, NT], BF16, tag="acc",
                                           bufs=2)
                            for ip in range(nk // 2):
                                st = ps_st.tile([P, 2, NT], F32, tag="st")
                                for half in range(2):
                                    ik = 2 * ip + half
                                    nc.tensor.matmul(
                                        st[:, half, :],
                                        lhsT=kT_res[:, b * S + ik * P:
                                                    b * S + (ik + 1) * P],
                                        rhs=qT_res[:, h, qsl],
                                        start=True, stop=True)
                                dst = acc if ip == 0 else pts.tile(
                                    [P, 2, NT], BF16, tag="pt", bufs=3)
                                r = 2 * ip - 4 * jq
                                if r >= 0:
                                    pr = pts.tile([P, 2, NT], BF16,
                                                  tag="pr", bufs=2)
                                    nc.scalar.activation(pr[:, :, :],
                                                         st[:, :, :],
                                                         AF.Exp, scale=SCALE)
                                    nc.vector.tensor_tensor(
                                        dst[:, :, :], pr[:, :, :],
                                        mask_sb[:, r:r + 2, :], op=OP.mult)
                                else:
                                    nc.scalar.activation(dst[:, :, :],
                                                         st[:, :, :],
                                                         AF.Exp, scale=SCALE)
                                if ip > 0:
                                    nc.vector.tensor_tensor(
                                        acc[:, :, :], acc[:, :, :],
                                        dst[:, :, :], op=OP.add)
                                for half in range(2):
                                    ik = 2 * ip + half
                                    nc.tensor.matmul(
                                        att_ps[:],
                                        lhsT=v_res[:, 16 * b + ik, :],
                                        rhs=dst[:, half, :],
                                        start=(ik == 0), stop=(ik == nk - 1))
                            den_ps = ps_o.tile([P, NT], F32, tag="o")
                            for half in range(2):
                                nc.tensor.matmul(den_ps[:], lhsT=ones_sb[:],
                                                 rhs=acc[:, half, :],
                                                 start=(half == 0),
                                                 stop=(half == 1))
                            rc = ep2.tile([P, NT], F32, tag="rc")
                            nc.vector.reciprocal_approx_fast(rc[:], den_ps[:])
                            nc.vector.tensor_tensor(at[:, h, :], att_ps[:],
                                                    rc[:], op=OP.mult)
                        if prev is not None:
                            emit_sweep(*prev)
                        prev = (b * SQT + jq, at)
                emit_sweep(*prev)

    nc.compile()
    return nc


def host_inputs(x, wq, wk, wv, wo, freqs_cos, freqs_sin):
    """Build the 8 per-core input maps from full inputs (pre-shuffled)."""
    bf16 = ml_dtypes.bfloat16
    x2 = np.asarray(x, dtype=np.float32).reshape(T, D)
    # xh[tt, kg, p, kc, n] = x2[tt*NT + n, kg*512 + kc*128 + p]
    xh = np.ascontiguousarray(
        x2.reshape(TT, NT, KG, 4, P).transpose(0, 2, 4, 3, 1)).astype(bf16)
    fc = np.asarray(freqs_cos, dtype=np.float32)
    fs = np.asarray(freqs_sin, dtype=np.float32)
    cc = np.repeat(fc.T, 2, axis=0)                         # [128, S]
    ss = np.repeat(fs.T, 2, axis=0)
    sgn = np.ones((P, 1), np.float32)
    sgn[0::2, 0] = -1.0
    cosE = np.ascontiguousarray(np.tile(cc, (1, B)).astype(np.float32))
    sinE = np.ascontiguousarray(np.tile(ss * sgn, (1, B)).astype(np.float32))
    perm_np = np.zeros((P, P), np.float32)
    for i in range(P):
        perm_np[i, i ^ 1] = 1.0
    perm_np = perm_np.astype(bf16)
    ident_np = np.eye(P, dtype=np.float32).astype(bf16)
    ones_np = np.ones((P, P), np.float32).astype(bf16)
    mk = np.zeros((P, NT // P, NT), np.float32)
    for r in range(NT // P):
        for p in range(P):
            mk[p, r, :] = np.where(np.arange(NT) >= 128 * r + p, 1.0, 0.0)
    mk = mk.astype(bf16)

    wq_f = np.asarray(wq, dtype=np.float32)
    wk_f = np.asarray(wk, dtype=np.float32)
    wv_f = np.asarray(wv, dtype=np.float32)
    wo_f = np.asarray(wo, dtype=np.float32)
    in_maps = []
    for g in range(8):
        wq_g = wq_f[:, g * NQ * HD:(g + 1) * NQ * HD]       # [D, 512]
        wk_g = wk_f[:, g * HD:(g + 1) * HD]                 # [D, 128]
        wv_g = wv_f[:, g * HD:(g + 1) * HD]
        wo_g = wo_f[g * NQ * HD:(g + 1) * NQ * HD, :]       # [512, D]
        # [P, DC, M] with element [p, c, m] = w[c*128 + p, m]
        wqh_np = np.ascontiguousarray(
            wq_g.reshape(DC, P, NQ * HD).transpose(1, 0, 2)).astype(bf16)
        wkh_np = np.ascontiguousarray(
            wk_g.reshape(DC, P, HD).transpose(1, 0, 2)).astype(bf16)
        wvh_np = np.ascontiguousarray(
            wv_g.reshape(DC, P, HD).transpose(1, 0, 2)).astype(bf16)
        # woh[p, j, dc, n] = wo_g[dc*128 + p, j*512 + n]
        woh_np = np.ascontiguousarray(
            wo_g.reshape(NQ, P, TT, NT).transpose(1, 2, 0, 3)).astype(bf16)
        in_maps.append({
            "xh": xh, "wqh": wqh_np, "wkh": wkh_np, "wvh": wvh_np,
            "woh": woh_np,
            "cosE": cosE, "sinE": sinE, "perm": perm_np, "ident": ident_np,
            "ones": ones_np, "maskneg": mk,
        })
    return in_maps


def combine_outputs(results):
    """Sum per-core partial^T and transpose back to [B, S, D]."""
    acc = results[0]["outT"].astype(np.float32)
    for r in results[1:]:
        acc += r["outT"].astype(np.float32)
    return np.ascontiguousarray(acc.T).reshape(B, S, D).astype(np.float32)


_NC = None


def kernel(x, wq, wk, wv, wo, freqs_cos, freqs_sin):
    """Full-input entry point: shards across 8 cores, runs, gathers."""
    global _NC
    from concourse.bass_utils import run_bass_kernel_spmd
    if _NC is None:
        _NC = build()
    in_maps = host_inputs(x, wq, wk, wv, wo, freqs_cos, freqs_sin)
    res = run_bass_kernel_spmd(_NC, in_maps, core_ids=list(range(8)),
                               trace=False)
    return combine_outputs(res.results)


# revision 18
# speedup vs baseline: 1.1611x; 1.0304x over previous
"""Self-contained Trainium2 Bass kernel for GQA causal self-attention.

Problem: x[2,2048,4096] @ wq/wk/wv (32 q-heads, 8 kv-heads, head_dim 128),
rope (precomputed freqs), causal softmax, GQA attention, wo projection.

Sharding: tensor-parallel across heads over 8 NeuronCores -- core g gets
kv-head g and q-heads 4g..4g+3 (wq/wk/wv column-sharded, wo row-sharded).
Each core computes a partial output projection; the host sums the 8
partials and transposes back (wo is row-parallel so partials just add).

v2: all tensors bf16 (halves DMA + enables FWL weight loads), Q^T and
attention outputs SBUF-resident (no DRAM spills), output projection fused
per 512-token block (software-pipelined one block behind attention), and
softmax denominators accumulated on the vector engine (bf16 adds) with
two small PE matmuls per (block, head) instead of one per k-chunk.
"""
import numpy as np
import ml_dtypes
import concourse.bacc as bacc
import concourse.mybir as mybir
import concourse.tile as tile

F32 = mybir.dt.float32
BF16 = mybir.dt.bfloat16
AF = mybir.ActivationFunctionType
OP = mybir.AluOpType

P = 128
B, S, D = 2, 2048, 4096
T = B * S            # 4096 tokens
HD = 128             # head dim
NQ = 4               # q heads per core
DC = D // P          # 32 contraction chunks
NT = 512             # free-dim tile
TT = T // NT         # 8 token tiles
SQT = S // NT        # 4 s_q tiles per batch
KG = 8               # k-chunk groups (4 chunks each) in phase 1
SCALE = 1.0 / float(np.sqrt(HD))


def build():
    nc = bacc.Bacc("TRN2", target_bir_lowering=False)
    # pre-shuffled inputs (see host_inputs)
    xh = nc.dram_tensor("xh", [TT, KG, P, 4, NT], BF16, kind="ExternalInput")
    wqh = nc.dram_tensor("wqh", [P, DC, NQ * HD], BF16, kind="ExternalInput")
    wkh = nc.dram_tensor("wkh", [P, DC, HD], BF16, kind="ExternalInput")
    wvh = nc.dram_tensor("wvh", [P, DC, HD], BF16, kind="ExternalInput")
    woh = nc.dram_tensor("woh", [P, TT, NQ, NT], BF16, kind="ExternalInput")
    cosE = nc.dram_tensor("cosE", [P, T], F32, kind="ExternalInput")
    sinE = nc.dram_tensor("sinE", [P, T], F32, kind="ExternalInput")
    perm = nc.dram_tensor("perm", [P, P], BF16, kind="ExternalInput")
    ident = nc.dram_tensor("ident", [P, P], BF16, kind="ExternalInput")
    ones = nc.dram_tensor("ones", [P, P], BF16, kind="ExternalInput")
    maskneg = nc.dram_tensor("maskneg", [P, NT // P, NT], BF16,
                             kind="ExternalInput")
    outT = nc.dram_tensor("outT", [D, T], BF16, kind="ExternalOutput")

    with tile.TileContext(nc) as tc:
        with (
            tc.tile_pool(name="res", bufs=1) as res,
            tc.tile_pool(name="outp", bufs=8) as outp,
        ):
            kT_res = res.tile([P, T], BF16)           # roped K^T, resident
            v_res = res.tile([P, DC, HD], BF16)       # V natural, resident
            qT_res = res.tile([P, NQ, T], BF16)       # roped Q^T, resident
            wo_sb = res.tile([P, TT, NQ, NT], BF16)   # wo, resident
            ones_sb = res.tile([P, P], BF16)
            nc.scalar.dma_start(ones_sb[:], ones[:, :])
            mask_sb = res.tile([P, NT // P, NT], BF16)
            nc.scalar.dma_start(mask_sb[:], maskneg[:, :, :])

            # ---------------- phase 1: projections + rope ----------------
            with (
                tc.tile_pool(name="c1", bufs=1) as c1,
                tc.tile_pool(name="xs", bufs=3) as xs,
                tc.tile_pool(name="epi", bufs=2) as epi,
                tc.tile_pool(name="ps_acc", bufs=7, space="PSUM") as ps_acc,
                tc.tile_pool(name="ps_misc", bufs=1, space="PSUM") as ps_misc,
            ):
                perm_sb = c1.tile([P, P], BF16)
                nc.scalar.dma_start(perm_sb[:], perm[:, :])
                # weights interleaved by k-group so tt0 matmuls start early
                wq_sb = c1.tile([P, DC, NQ * HD], BF16)
                wk_sb = c1.tile([P, DC, HD], BF16)
                wv_sb = c1.tile([P, DC, HD], BF16)

                # weights on the ACT HWDGE ring (separate FIFO from the x
                # stream on SP), batched coarser as the deadline recedes
                for ksl in (slice(0, 4), slice(4, 8), slice(8, 16),
                            slice(16, 32)):
                    nc.scalar.dma_start(wq_sb[:, ksl, :], wqh[:, ksl, :])
                    nc.scalar.dma_start(wk_sb[:, ksl, :], wkh[:, ksl, :])
                    nc.scalar.dma_start(wv_sb[:, ksl, :], wvh[:, ksl, :])

                for tt in range(TT):
                    tsl = slice(tt * NT, (tt + 1) * NT)
                    if tt == 2:
                        # deferred past the ramp so it doesn't steal HBM
                        # bandwidth from the first x/weight tiles
                        nc.scalar.dma_start(wo_sb[:], woh[:, :, :, :])
                    cos_t = xs.tile([P, NT], F32, tag="cos", bufs=2)
                    sin_t = xs.tile([P, NT], F32, tag="sin", bufs=2)
                    nc.scalar.dma_start(cos_t[:], cosE[:, tsl])
                    nc.scalar.dma_start(sin_t[:], sinE[:, tsl])

                    accs = [ps_acc.tile([P, NT], F32, tag="acc",
                                        name=f"acc{tt}_{oc}")
                            for oc in range(6)]
                    v_ps = accs[5]
                    for kg in range(KG):
                        xt = xs.tile([P, 4, NT], BF16, tag="xt", bufs=3)
                        if tt == 0 and kg == 0:
                            # split so the first matmuls start on slice 0
                            # while the DMA engines are still ramping
                            for kc in range(4):
                                nc.sync.dma_start(xt[:, kc, :],
                                                  xh[tt, kg, :, kc, :])
                        else:
                            nc.sync.dma_start(xt[:], xh[tt, kg, :, :, :])
                        for kc in range(4):
                            k = kg * 4 + kc
                            for oc in range(6):
                                if oc < 4:
                                    lhsT = wq_sb[:, k, oc * P:(oc + 1) * P]
                                elif oc == 4:
                                    lhsT = wk_sb[:, k, :]
                                else:
                                    lhsT = wv_sb[:, k, :]
                                nc.tensor.matmul(accs[oc][:], lhsT=lhsT,
                                                 rhs=xt[:, kc, :],
                                                 start=(k == 0),
                                                 stop=(k == DC - 1))

                    # rope epilogue for Q (oc 0..3) and K (oc 4)
                    for oc in range(5):
                        acc = accs[oc]
                        sb_r = epi.tile([P, NT], BF16, tag="sbr")
                        nc.scalar.copy(sb_r[:], acc[:])
                        sw = ps_misc.tile([P, NT], F32, tag="misc", name="sw")
                        nc.tensor.matmul(sw[:], lhsT=perm_sb[:], rhs=sb_r[:],
                                         start=True, stop=True)
                        t1 = epi.tile([P, NT], F32, tag="t1")
                        nc.vector.tensor_tensor(t1[:], acc[:], cos_t[:],
                                                op=OP.mult)
                        t2 = epi.tile([P, NT], F32, tag="t2")
                        nc.vector.tensor_tensor(t2[:], sw[:], sin_t[:],
                                                op=OP.mult)
                        if oc < 4:
                            nc.vector.tensor_tensor(qT_res[:, oc, tsl],
                                                    t1[:], t2[:], op=OP.add)
                        else:
                            nc.vector.tensor_tensor(kT_res[:, tsl],
                                                    t1[:], t2[:], op=OP.add)

                    # V epilogue: V^T chunks -> DMA xbar transpose -> V
                    vsb = epi.tile([P, NT], BF16, tag="sbr")
                    nc.scalar.copy(vsb[:], v_ps[:])
                    for c in range(NT // P):
                        nc.sync.dma_start(v_res[:, 4 * tt + c, :],
                                          vsb[:, c * P:(c + 1) * P],
                                          transpose=True)

            # ---------- phase 2+3 fused: attention + output proj ----------
            with (
                tc.tile_pool(name="pts", bufs=3) as pts,
                tc.tile_pool(name="ep2", bufs=2) as ep2,
                tc.tile_pool(name="ats", bufs=2) as ats,
                tc.tile_pool(name="ps_st", bufs=2, space="PSUM") as ps_st,
                tc.tile_pool(name="ps_att", bufs=2, space="PSUM") as ps_att,
                tc.tile_pool(name="ps_o", bufs=2, space="PSUM") as ps_o,
            ):
                def emit_oc(jt, at, oc):
                    jsl = slice(jt * NT, (jt + 1) * NT)
                    o_ps = ps_o.tile([P, NT], F32, tag="o")
                    wj, wn = oc // 4, (oc % 4) * P
                    for dc in range(NQ):
                        nc.tensor.matmul(o_ps[:],
                                         lhsT=wo_sb[:, wj, dc, wn:wn + P],
                                         rhs=at[:, dc, :],
                                         start=(dc == 0),
                                         stop=(dc == NQ - 1))
                    osb = outp.tile([P, NT], BF16, tag="ot")
                    if oc % 2 == 0:
                        nc.scalar.copy(osb[:], o_ps[:])
                    else:
                        nc.vector.tensor_copy(osb[:], o_ps[:])
                    nc.sync.dma_start(outT[oc * P:(oc + 1) * P, jsl],
                                      osb[:])

                prev = None
                # jq descending: the first block (no prior sweep to hide
                # exp latency behind) gets the deepest matmul pipeline
                for b in range(B):
                    for jq in sorted(range(SQT), reverse=(b == 0)):
                        nk = 4 * (jq + 1)
                        qsl = slice(b * S + jq * NT, b * S + (jq + 1) * NT)
                        at = ats.tile([P, NQ, NT], BF16, tag="at")
                        # previous block's output-projection matmuls are
                        # interleaved between score and attnV pairs: they
                        # fill the PE while ScalarE runs exp, instead of
                        # the PE FIFO stalling on the exp dependency
                        pend = list(range(DC)) if prev is not None else []
                        rem_pairs = NQ * (nk // 2)
                        for h in range(NQ):
                            att_ps = ps_att.tile([P, NT], F32, tag="att")
                            acc = pts.tile([P, 2, NT], BF16, tag="acc",
                                           bufs=2)
                            for ip in range(nk // 2):
                                st = ps_st.tile([P, 2, NT], F32, tag="st")
                                for half in range(2):
                                    ik = 2 * ip + half
                                    nc.tensor.matmul(
                                        st[:, half, :],
                                        lhsT=kT_res[:, b * S + ik * P:
                                                    b * S + (ik + 1) * P],
                                        rhs=qT_res[:, h, qsl],
                                        start=True, stop=True)
                                dst = acc if ip == 0 else pts.tile(
                                    [P, 2, NT], BF16, tag="pt", bufs=4)
                                r = 2 * ip - 4 * jq
                                if r >= 0:
                                    pr = pts.tile([P, 2, NT], BF16,
                                                  tag="pr", bufs=3)
                                    nc.scalar.activation(pr[:, :, :],
                                                         st[:, :, :],
                                                         AF.Exp, scale=SCALE)
                                    nc.vector.tensor_tensor(
                                        dst[:, :, :], pr[:, :, :],
                                        mask_sb[:, r:r + 2, :], op=OP.mult)
                                else:
                                    nc.scalar.activation(dst[:, :, :],
                                                         st[:, :, :],
                                                         AF.Exp, scale=SCALE)
                                if ip > 0:
                                    nc.vector.tensor_tensor(
                                        acc[:, :, :], acc[:, :, :],
                                        dst[:, :, :], op=OP.add)
                                if pend:
                                    take = -(-len(pend) // rem_pairs)
                                    for oc in pend[:take]:
                                        emit_oc(prev[0], prev[1], oc)
                                    pend = pend[take:]
                                rem_pairs -= 1
                                for half in range(2):
                                    ik = 2 * ip + half
                                    nc.tensor.matmul(
                                        att_ps[:],
                                        lhsT=v_res[:, 16 * b + ik, :],
                                        rhs=dst[:, half, :],
                                        start=(ik == 0), stop=(ik == nk - 1))
                            den_ps = ps_o.tile([P, NT], F32, tag="o")
                            for half in range(2):
                                nc.tensor.matmul(den_ps[:], lhsT=ones_sb[:],
                                                 rhs=acc[:, half, :],
                                                 start=(half == 0),
                                                 stop=(half == 1))
                            rc = ep2.tile([P, NT], F32, tag="rc")
                            nc.vector.reciprocal_approx_fast(rc[:], den_ps[:])
                            nc.vector.tensor_tensor(at[:, h, :], att_ps[:],
                                                    rc[:], op=OP.mult)
                        prev = (b * SQT + jq, at)
                for oc in range(DC):
                    emit_oc(prev[0], prev[1], oc)

    nc.compile()
    return nc


def host_inputs(x, wq, wk, wv, wo, freqs_cos, freqs_sin):
    """Build the 8 per-core input maps from full inputs (pre-shuffled)."""
    bf16 = ml_dtypes.bfloat16
    x2 = np.asarray(x, dtype=np.float32).reshape(T, D)
    # xh[tt, kg, p, kc, n] = x2[tt*NT + n, kg*512 + kc*128 + p]
    xh = np.ascontiguousarray(
        x2.reshape(TT, NT, KG, 4, P).transpose(0, 2, 4, 3, 1)).astype(bf16)
    fc = np.asarray(freqs_cos, dtype=np.float32)
    fs = np.asarray(freqs_sin, dtype=np.float32)
    cc = np.repeat(fc.T, 2, axis=0)                         # [128, S]
    ss = np.repeat(fs.T, 2, axis=0)
    sgn = np.ones((P, 1), np.float32)
    sgn[0::2, 0] = -1.0
    cosE = np.ascontiguousarray(np.tile(cc, (1, B)).astype(np.float32))
    sinE = np.ascontiguousarray(np.tile(ss * sgn, (1, B)).astype(np.float32))
    perm_np = np.zeros((P, P), np.float32)
    for i in range(P):
        perm_np[i, i ^ 1] = 1.0
    perm_np = perm_np.astype(bf16)
    ident_np = np.eye(P, dtype=np.float32).astype(bf16)
    ones_np = np.ones((P, P), np.float32).astype(bf16)
    mk = np.zeros((P, NT // P, NT), np.float32)
    for r in range(NT // P):
        for p in range(P):
            mk[p, r, :] = np.where(np.arange(NT) >= 128 * r + p, 1.0, 0.0)
    mk = mk.astype(bf16)

    wq_f = np.asarray(wq, dtype=np.float32)
    wk_f = np.asarray(wk, dtype=np.float32)
    wv_f = np.asarray(wv, dtype=np.float32)
    wo_f = np.asarray(wo, dtype=np.float32)
    in_maps = []
    for g in range(8):
        wq_g = wq_f[:, g * NQ * HD:(g + 1) * NQ * HD]       # [D, 512]
        wk_g = wk_f[:, g * HD:(g + 1) * HD]                 # [D, 128]
        wv_g = wv_f[:, g * HD:(g + 1) * HD]
        wo_g = wo_f[g * NQ * HD:(g + 1) * NQ * HD, :]       # [512, D]
        # [P, DC, M] with element [p, c, m] = w[c*128 + p, m]
        wqh_np = np.ascontiguousarray(
            wq_g.reshape(DC, P, NQ * HD).transpose(1, 0, 2)).astype(bf16)
        wkh_np = np.ascontiguousarray(
            wk_g.reshape(DC, P, HD).transpose(1, 0, 2)).astype(bf16)
        wvh_np = np.ascontiguousarray(
            wv_g.reshape(DC, P, HD).transpose(1, 0, 2)).astype(bf16)
        # woh[p, j, dc, n] = wo_g[dc*128 + p, j*512 + n]
        woh_np = np.ascontiguousarray(
            wo_g.reshape(NQ, P, TT, NT).transpose(1, 2, 0, 3)).astype(bf16)
        in_maps.append({
            "xh": xh, "wqh": wqh_np, "wkh": wkh_np, "wvh": wvh_np,
            "woh": woh_np,
            "cosE": cosE, "sinE": sinE, "perm": perm_np, "ident": ident_np,
            "ones": ones_np, "maskneg": mk,
        })
    return in_maps


def combine_outputs(results):
    """Sum per-core partial^T and transpose back to [B, S, D]."""
    acc = results[0]["outT"].astype(np.float32)
    for r in results[1:]:
        acc += r["outT"].astype(np.float32)
    return np.ascontiguousarray(acc.T).reshape(B, S, D).astype(np.float32)


_NC = None


def kernel(x, wq, wk, wv, wo, freqs_cos, freqs_sin):
    """Full-input entry point: shards across 8 cores, runs, gathers."""
    global _NC
    from concourse.bass_utils import run_bass_kernel_spmd
    if _NC is None:
        _NC = build()
    in_maps = host_inputs(x, wq, wk, wv, wo, freqs_cos, freqs_sin)
    res = run_bass_kernel_spmd(_NC, in_maps, core_ids=list(range(8)),
                               trace=False)
    return combine_outputs(res.results)
